# revision 16
# baseline (speedup 1.0000x reference)
"""Trainium2 Bass kernel for nn_DZSpecimenClfToy.

Reference computation (per batch item b, B=8, one NeuronCore each):
  1. tv = bilinear_resize(topview[b], (3,64,64) -> (3,4,4))   # fixed 2x2 avg of rows/cols {7,8},{23,24},{39,40},{55,56}
  2. coords = sigmoid(tv.flat @ W1.T + b1).reshape(N,2)       # N=4096
  3. patch top-left tl = coords*2043; all 16 output px of a 4x4
     patch share one bilinear fraction pair -> 5x5 pixel support
  4. out[b] = bilinear_crops.flat @ W2.T + b2                 # [2]

Sharding: data-parallel over batch across 8 cores; weights replicated.

Gather strategy: the host uploads the search view (fp16) in an
overlapped-band layout: 511 bands of 8 rows (stride 4), each stored
[col][row_in_band][ch]. A patch's 5x5x3 support is then one contiguous
111-element run starting at triple index b*16384 + c0*8 + s (b=r0//4,
s=r0%4); we gather 128 elements (256B) per patch. The indirect DMA's
offset AP carries 8 indices per partition, so 4096 patches = 4
instructions of 1024 descriptors each, pipelined with the bilinear
interpolation + classifier contraction of earlier chunks.
"""
import functools
from contextlib import ExitStack

import numpy as np

import concourse.bass as bass
import concourse.tile as tile
from concourse import bacc, mybir
import concourse.bass_utils as bass_utils
from concourse.bass import IndirectOffsetOnAxis

F32 = mybir.dt.float32
F16 = mybir.dt.float16
I32 = mybir.dt.int32
ALU = mybir.AluOpType
ACT = mybir.ActivationFunctionType
AX = mybir.AxisListType

B = 8          # batch == number of cores
H = W = 2048   # search view height/width
N = 4096       # patches per item
PS = 4         # patch size
NCLS = 2       # classes
P = 128        # partitions
TPP = N // P   # patches per partition = 32

NBAND = 511            # bands of 8 rows, stride 4: rows 4b..4b+7
BANDTRIP = W * 8       # pixel-triples per band = 16384
PADTRIP = 64           # svb tail pad: SEG overreads past the last band
SEG = 128              # gathered elements per patch (111 used, 256B fp16)
MAGIC = 8388608.0      # 2**23

NCH = 4                # gather/compute chunks
TPC = TPP // NCH       # patches per partition per chunk = 8

MULTI_IDX_GATHER = False   # one indirect DMA per chunk (vs one per patch slot)
USE_TTR = False            # fused tensor_tensor_reduce classifier
MM_BCAST = True           # matmul broadcast of flat (vs DRAM bounce)


def build_program(num_devices: int, svh: int, svw: int, debug: bool = False):
    pad = float(svh - 1 - PS)  # 2043
    assert svh == H and svw == W, (svh, svw)

    nc = bacc.Bacc("TRN2", target_bir_lowering=False, debug=False,
                   enable_asserts=False, num_devices=num_devices)

    tv = nc.dram_tensor("tv", [3, 64, 64], F32, kind="ExternalInput").ap()
    svb = nc.dram_tensor("svb", [NBAND * BANDTRIP + PADTRIP, 3], F16,
                         kind="ExternalInput").ap()
    w1 = nc.dram_tensor("W1", [2 * N, 48], F32, kind="ExternalInput").ap()
    b1 = nc.dram_tensor("b1", [2 * N], F32, kind="ExternalInput").ap()
    w2 = nc.dram_tensor("W2p", [NCLS, N * PS * PS * 3], F16, kind="ExternalInput").ap()
    b2 = nc.dram_tensor("b2", [NCLS], F32, kind="ExternalInput").ap()
    out = nc.dram_tensor("out", [1, NCLS], F32, kind="ExternalOutput").ap()

    dbg = {}
    if debug:
        dbg["idx"] = nc.dram_tensor("dbg_idx", [P, TPP], I32, kind="ExternalOutput").ap()
        dbg["S"] = nc.dram_tensor("dbg_S", [P, TPP * SEG], F16, kind="ExternalOutput").ap()
        dbg["U"] = nc.dram_tensor("dbg_U", [P, TPP * 48], F16, kind="ExternalOutput").ap()

    with tile.TileContext(nc) as tc:
        with ExitStack() as ctx:
            pool = ctx.enter_context(tc.tile_pool(name="main", bufs=1))

            # ---- input DMAs -------------------------------------------------
            # Topview rows {7,8},{23,24},{39,40},{55,56}: each pair is 128
            # contiguous floats starting at row 7 of each 16-row group.
            A = pool.tile([1, 1536], F32)          # [(c,k), r01*64]
            tv_sel = tv.rearrange("c (k s) w -> c k (s w)", s=16)[:, :, 7 * 64:9 * 64]
            nc.sync.dma_start(A[:].rearrange("p (c k x) -> p c k x", c=3, k=4),
                              tv_sel.unsqueeze(0))

            # W1 in two halves: rows g = p*64+j, half h covers j in [32h, 32h+32)
            W1sb = pool.tile([P, 64 * 48], F32)    # row g=p*64+j at [p, j*48:...]
            w1v = w1.rearrange("(p j) k -> p (j k)", p=P)
            nc.sync.dma_start(W1sb[:, :32 * 48], w1v[:, :32 * 48])
            nc.sync.dma_start(W1sb[:, 32 * 48:], w1v[:, 32 * 48:])

            b1sb = pool.tile([P, 64], F32)
            nc.scalar.dma_start(b1sb[:], b1.rearrange("(p j) -> p j", p=P))

            b2sb = pool.tile([1, NCLS], F32)
            nc.scalar.dma_start(b2sb[:], b2.unsqueeze(0))

            W2sb = pool.tile([P, NCLS * 1536], F16)  # [p, c*1536+x] = W2p[c, p*1536+x]
            nc.scalar.dma_start(W2sb[:].rearrange("p (c x) -> p c x", c=NCLS),
                                w2.rearrange("c (p x) -> p c x", p=P))

            # ---- topview 64x64 -> 4x4 resize, flatten, scale ---------------
            V = pool.tile([1, 768], F32)           # [(c,k), 64] row-pair sums
            A4 = A[:].rearrange("p (ck r w) -> p ck r w", ck=12, r=2)
            nc.vector.tensor_add(V[:].rearrange("p (ck w) -> p ck w", ck=12),
                                 A4[:, :, 0, :], A4[:, :, 1, :])
            F48 = pool.tile([1, 48], F32)
            V4 = V[:].rearrange("p (ck g s) -> p ck g s", ck=12, g=4)
            nc.vector.tensor_add(F48[:].rearrange("p (ck g) -> p ck g", ck=12),
                                 V4[:, :, :, 7], V4[:, :, :, 8])
            flatF = pool.tile([1, 48], F32)
            nc.vector.tensor_scalar_mul(flatF[:], F48[:], 0.25)

            # broadcast flat to all partitions via PE outer product with ones
            ppool = ctx.enter_context(tc.tile_pool(name="ps", bufs=1, space="PSUM"))
            if MM_BCAST:
                ones1 = pool.tile([1, P], F32)
                nc.vector.memset(ones1[:], 1.0)
                flatb = ppool.tile([P, 48], F32)
                nc.tensor.matmul(out=flatb[:], lhsT=ones1[:], rhs=flatF[:],
                                 start=True, stop=True)
            else:
                dram_pool = ctx.enter_context(
                    tc.tile_pool(name="dram", bufs=1, space="DRAM"))
                fdram = dram_pool.tile([1, 48], F32)
                nc.sync.dma_start(fdram[:], flatF[:])
                flatb = pool.tile([P, 48], F32)
                nc.sync.dma_start(flatb[:], fdram[:].to_broadcast((P, 48)))

            # ---- coords = sigmoid(W1 @ flat + b1), [128, 64], per half -----
            mul1 = pool.tile([P, 64 * 48], F32)
            pre = pool.tile([P, 64], F32)
            sg = pool.tile([P, 64], F32)
            for h in range(2):
                js = slice(h * 32 * 48, (h + 1) * 32 * 48)
                nc.vector.tensor_mul(
                    mul1[:, js].rearrange("p (j k) -> p j k", j=32),
                    W1sb[:, js].rearrange("p (j k) -> p j k", j=32),
                    flatb[:].unsqueeze(1).to_broadcast((P, 32, 48)))
                nc.vector.reduce_sum(
                    pre[:, h * 32:(h + 1) * 32].unsqueeze(2),
                    mul1[:, js].rearrange("p (j k) -> p j k", j=32),
                    axis=AX.X)
                nc.vector.tensor_add(pre[:, h * 32:(h + 1) * 32],
                                     pre[:, h * 32:(h + 1) * 32],
                                     b1sb[:, h * 32:(h + 1) * 32])
                nc.scalar.activation(sg[:, h * 32:(h + 1) * 32],
                                     pre[:, h * 32:(h + 1) * 32], ACT.Sigmoid)

            # ---- patch corners, gather indices, fractions (per half) -------
            # tl = sigmoid * pad (the +-ps//2 cancels); idx = b*16380 + r0 + 8*c0
            tl64 = pool.tile([P, 64], F32)
            rnd64 = pool.tile([P, 64], F32)
            gt64 = pool.tile([P, 64], F32)
            c064 = pool.tile([P, 64], F32)
            fr64 = pool.tile([P, 64], F32)
            bq = pool.tile([P, TPP], F32)
            rndb = pool.tile([P, TPP], F32)
            gtb = pool.tile([P, TPP], F32)
            bf = pool.tile([P, TPP], F32)
            t2 = pool.tile([P, TPP], F32)
            t1 = pool.tile([P, TPP], F32)
            idxf = pool.tile([P, TPP], F32)
            idxi = pool.tile([P, TPP], I32)
            fr16 = pool.tile([P, 64], F16)

            def half_views(t_, h):
                return t_[:, h * 32:(h + 1) * 32]

            def qv(t_, h):  # [128, 16] views of per-coordinate tiles
                return t_[:, h * 16:(h + 1) * 16]

            for h in range(2):
                hv = functools.partial(half_views, h=h)
                # floor via round-to-nearest + correction (values >= 0)
                nc.vector.tensor_scalar_mul(hv(tl64), hv(sg), pad)
                nc.vector.tensor_scalar(hv(rnd64), hv(tl64), MAGIC, MAGIC,
                                        op0=ALU.add, op1=ALU.subtract)
                nc.vector.tensor_tensor(hv(gt64), hv(rnd64), hv(tl64), op=ALU.is_gt)
                nc.vector.tensor_sub(hv(c064), hv(rnd64), hv(gt64))

                c2 = hv(c064).rearrange("p (t two) -> p t two", two=2)
                r0f, c0f = c2[:, :, 0], c2[:, :, 1]
                # band = floor(r0/4); idx = band*16380 + r0 + 8*c0
                nc.vector.tensor_scalar_mul(qv(bq, h), r0f, 0.25)
                nc.vector.tensor_scalar(qv(rndb, h), qv(bq, h), MAGIC, MAGIC,
                                        op0=ALU.add, op1=ALU.subtract)
                nc.vector.tensor_tensor(qv(gtb, h), qv(rndb, h), qv(bq, h), op=ALU.is_gt)
                nc.vector.tensor_sub(qv(bf, h), qv(rndb, h), qv(gtb, h))
                nc.vector.scalar_tensor_tensor(qv(t2, h), c0f, 8.0, r0f,
                                               op0=ALU.mult, op1=ALU.add)
                nc.vector.tensor_scalar(qv(t1, h), qv(bf, h), float(BANDTRIP - 4), MAGIC,
                                        op0=ALU.mult, op1=ALU.add)
                nc.vector.tensor_add(qv(idxf, h), qv(t1, h), qv(t2, h))
                nc.vector.tensor_single_scalar(qv(idxi, h), qv(idxf, h).bitcast(I32),
                                               0x007FFFFF, op=ALU.bitwise_and)
            if debug:
                nc.sync.dma_start(dbg["idx"], idxi[:])

            # fractions (needed only after the first gathers are issued)
            for h in range(2):
                hv = functools.partial(half_views, h=h)
                nc.vector.tensor_sub(hv(fr64), hv(tl64), hv(c064))
                nc.scalar.copy(hv(fr16), hv(fr64))

            fr2 = fr16[:].rearrange("p (t two) -> p t two", two=2)

            # ---- gather + bilinear + classifier, chunked -------------------
            S = pool.tile([P, TPP * SEG], F16)
            D1 = pool.tile([P, TPP * 60], F16)
            M1 = pool.tile([P, TPP * 60], F16)
            T = pool.tile([P, TPP * 60], F16)
            D2 = pool.tile([P, TPP * 48], F16)
            M2 = pool.tile([P, TPP * 48], F16)
            U = pool.tile([P, TPP * 48], F16)
            prod = pool.tile([P, NCLS * 1536], F16)
            r2cls = pool.tile([P, NCLS * NCH], F32)

            Sv = S[:].rearrange("p (t x) -> p t x", t=TPP)
            W2v = W2sb[:].rearrange("p (c t x) -> p c t x", c=NCLS, t=TPP)
            prodv = prod[:].rearrange("p (c t x) -> p c t x", c=NCLS, t=TPP)

            if MULTI_IDX_GATHER:
                for c in range(NCH):
                    ts = slice(c * TPC, (c + 1) * TPC)
                    nc.gpsimd.indirect_dma_start(
                        out=S[:, c * TPC * SEG:(c + 1) * TPC * SEG],
                        out_offset=None,
                        in_=svb,
                        in_offset=IndirectOffsetOnAxis(ap=idxi[:, ts], axis=0),
                    )
            else:
                for t in range(TPP):
                    nc.gpsimd.indirect_dma_start(
                        out=S[:, t * SEG:(t + 1) * SEG],
                        out_offset=None,
                        in_=svb,
                        in_offset=IndirectOffsetOnAxis(ap=idxi[:, t:t + 1], axis=0),
                    )

            if debug:
                nc.sync.dma_start(dbg["S"], S[:])

            for c in range(NCH):
                ts = slice(c * TPC, (c + 1) * TPC)

                def seg_view(off):
                    # [p, t, d(5 cols, stride 24), 12 = (i,c)] at offset off
                    return Sv[:, ts, off:off + 120].rearrange(
                        "p t (d e) -> p t d e", d=5)[:, :, :, 0:12]

                frb = fr2[:, ts, 0].unsqueeze(2).unsqueeze(3).to_broadcast((P, TPC, 5, 12))
                fcb = fr2[:, ts, 1].unsqueeze(2).unsqueeze(3).to_broadcast((P, TPC, 4, 12))

                def c60(t_):
                    return t_[:, c * TPC * 60:(c + 1) * TPC * 60].rearrange(
                        "p (t d e) -> p t d e", t=TPC, d=5)

                def c48(t_):
                    return t_[:, c * TPC * 48:(c + 1) * TPC * 48].rearrange(
                        "p (t d e) -> p t d e", t=TPC, d=4)

                # row interp: T[d, i] = S[d, i] + fr*(S[d, i+1] - S[d, i])
                nc.vector.tensor_sub(c60(D1), seg_view(3), seg_view(0))
                nc.vector.tensor_mul(c60(M1), c60(D1), frb)
                nc.vector.tensor_add(c60(T), c60(M1), seg_view(0))

                # col interp: U[j, i] = T(d=j) + fc*(T(d=j+1) - T(d=j))
                Tc = T[:, c * TPC * 60:(c + 1) * TPC * 60].rearrange(
                    "p (t x) -> p t x", t=TPC)
                T0 = Tc[:, :, 0:48].rearrange("p t (d e) -> p t d e", d=4)
                T12 = Tc[:, :, 12:60].rearrange("p t (d e) -> p t d e", d=4)
                nc.vector.tensor_sub(c48(D2), T12, T0)
                nc.vector.tensor_mul(c48(M2), c48(D2), fcb)
                nc.vector.tensor_add(c48(U), c48(M2), T0)

                # classifier partial: r2cls[p, k, c] = sum(W2[k] * U) over chunk
                Uc = U[:, c * TPC * 48:(c + 1) * TPC * 48]
                for k in range(NCLS):
                    if USE_TTR:
                        nc.vector.tensor_tensor_reduce(
                            out=prodv[:, k, ts, :].rearrange("p t x -> p (t x)"),
                            in0=W2v[:, k, ts, :].rearrange("p t x -> p (t x)"),
                            in1=Uc,
                            scale=1.0,
                            scalar=0.0,
                            op0=ALU.mult,
                            op1=ALU.add,
                            accum_out=r2cls[:, k * NCH + c:k * NCH + c + 1],
                        )
                    else:
                        pk = prodv[:, k, ts, :].rearrange("p t x -> p (t x)")
                        nc.vector.tensor_mul(
                            pk, W2v[:, k, ts, :].rearrange("p t x -> p (t x)"), Uc)
                        nc.vector.reduce_sum(
                            r2cls[:, k * NCH + c:k * NCH + c + 1].unsqueeze(2),
                            pk.unsqueeze(1),
                            axis=AX.X)

            if debug:
                nc.sync.dma_start(dbg["U"], U[:])

            # ---- finalize: sum chunks, reduce over partitions, + b2 --------
            r2 = pool.tile([P, NCLS], F32)
            nc.vector.reduce_sum(r2[:].unsqueeze(2),
                                 r2cls[:].rearrange("p (c n) -> p c n", c=NCLS),
                                 axis=AX.X)
            ones = pool.tile([P, 1], F32)
            nc.vector.memset(ones[:], 1.0)
            osum = ppool.tile([1, NCLS], F32)
            nc.tensor.matmul(out=osum[:], lhsT=ones[:], rhs=r2[:], start=True, stop=True)
            ofin = pool.tile([1, NCLS], F32)
            nc.vector.tensor_add(ofin[:], osum[:], b2sb[:])
            nc.sync.dma_start(out, ofin[:])

    nc.compile()
    return nc


@functools.lru_cache(maxsize=2)
def _compiled(num_devices: int, svh: int, svw: int, debug: bool = False):
    return build_program(num_devices, svh, svw, debug)


def band_layout(img: np.ndarray) -> np.ndarray:
    """[2048, 2048, 3] f32 -> [511*16384 + pad, 3] fp16 bands."""
    sw = np.lib.stride_tricks.sliding_window_view(img, 8, axis=0)  # [2041, 2048, 3, 8]
    sb = sw[::4]                                                   # [511, 2048, 3, 8]
    flat = np.ascontiguousarray(
        sb.transpose(0, 1, 3, 2), dtype=np.float16).reshape(-1, 3)
    return np.concatenate([flat, np.zeros((PADTRIP, 3), np.float16)], axis=0)


def permute_w2(W2: np.ndarray) -> np.ndarray:
    """Reorder per-patch (i, j, c) -> (j, i, c) to match the kernel's U layout."""
    return np.ascontiguousarray(
        W2.reshape(NCLS, N, PS, PS, 3).transpose(0, 1, 3, 2, 4)).reshape(NCLS, -1)


def make_in_maps(topview, search_views, W1, b1, W2, b2):
    W1 = np.ascontiguousarray(W1, np.float32)
    b1 = np.ascontiguousarray(b1, np.float32)
    W2p = permute_w2(np.asarray(W2, np.float32)).astype(np.float16)
    b2 = np.ascontiguousarray(b2, np.float32)
    return [{
        "tv": np.ascontiguousarray(topview[i], np.float32),
        "svb": band_layout(np.asarray(search_views[i], np.float32)),
        "W1": W1, "b1": b1, "W2p": W2p, "b2": b2,
    } for i in range(topview.shape[0])]


def kernel(topview, search_views, W1, b1, W2, b2, svh, svw):
    svh, svw = int(svh), int(svw)
    nc = _compiled(B, svh, svw)
    in_maps = make_in_maps(topview, search_views, W1, b1, W2, b2)
    res = bass_utils.run_bass_kernel_spmd(nc, in_maps, core_ids=list(range(B)))
    return np.concatenate([res.results[i]["out"] for i in range(B)], axis=0)


# revision 17
# speedup vs baseline: 1.0365x; 1.0365x over previous
"""Trainium2 Bass kernel for nn_DZSpecimenClfToy.

Reference computation (per batch item b, B=8, one NeuronCore each):
  1. tv = bilinear_resize(topview[b], (3,64,64) -> (3,4,4))
  2. coords = sigmoid(tv.flat @ W1.T + b1).reshape(N,2)       # N=4096
  3. 4x4 patches bilinearly sampled at coords*2043 (5x5 px support)
  4. out[b] = patches.flat @ W2.T + b2                        # [2]

Sharding: data-parallel over batch across 8 cores; weights replicated.

Gather: host uploads the search view (fp16) in an overlapped-band layout
(511 bands of 8 rows at stride 4, each [col][row][ch]), so a patch's
5x5x3 support is one contiguous run at triple index b*16384 + c0*8 + s.
The HW indirect DMA consumes ONE offset per partition per instruction,
so the 4096 patches take 32 gathers of [128 x SEG]. The coords pipeline
is split in quarters so the gather train starts as early as possible,
and the bilinear + classifier (fp16, fused mul+accum) hide under it.
"""
import functools
from contextlib import ExitStack

import numpy as np

import concourse.bass as bass
import concourse.tile as tile
from concourse import bacc, mybir
import concourse.bass_utils as bass_utils
from concourse.bass import IndirectOffsetOnAxis

F32 = mybir.dt.float32
F16 = mybir.dt.float16
I32 = mybir.dt.int32
ALU = mybir.AluOpType
ACT = mybir.ActivationFunctionType
AX = mybir.AxisListType

B = 8          # batch == number of cores
H = W = 2048   # search view height/width
N = 4096       # patches per item
PS = 4         # patch size
NCLS = 2       # classes
P = 128        # partitions
TPP = N // P   # patches per partition = 32

NBAND = 511            # bands of 8 rows, stride 4: rows 4b..4b+7
BANDTRIP = W * 8       # pixel-triples per band = 16384
PADTRIP = 128          # svb tail pad: SEG overreads past the last band
SEG = 128              # gathered fp16 elements per patch (111 used)
MAGIC = 8388608.0      # 2**23

NQ = 4                 # coords pipeline quarters == gather/compute chunks
TPC = TPP // NQ        # patches per partition per chunk = 8
JPQ = 64 // NQ         # gates per partition per quarter = 16


def build_program(num_devices: int, svh: int, svw: int, debug: bool = False):
    pad = float(svh - 1 - PS)  # 2043
    assert svh == H and svw == W, (svh, svw)

    nc = bacc.Bacc("TRN2", target_bir_lowering=False, debug=False,
                   enable_asserts=False, num_devices=num_devices)

    tv = nc.dram_tensor("tv", [3, 64, 64], F32, kind="ExternalInput").ap()
    svb = nc.dram_tensor("svb", [NBAND * BANDTRIP + PADTRIP, 3], F16,
                         kind="ExternalInput").ap()
    w1 = nc.dram_tensor("W1", [2 * N, 48], F32, kind="ExternalInput").ap()
    b1 = nc.dram_tensor("b1", [2 * N], F32, kind="ExternalInput").ap()
    w2 = nc.dram_tensor("W2p", [NCLS, N * PS * PS * 3], F16, kind="ExternalInput").ap()
    b2 = nc.dram_tensor("b2", [NCLS], F32, kind="ExternalInput").ap()
    out = nc.dram_tensor("out", [1, NCLS], F32, kind="ExternalOutput").ap()

    dbg = {}
    if debug:
        dbg["idx"] = nc.dram_tensor("dbg_idx", [P, TPP], I32, kind="ExternalOutput").ap()
        dbg["S"] = nc.dram_tensor("dbg_S", [P, TPP * SEG], F16, kind="ExternalOutput").ap()

    with tile.TileContext(nc) as tc:
        with ExitStack() as ctx:
            pool = ctx.enter_context(tc.tile_pool(name="main", bufs=1))

            # ---- input DMAs (order matters: tv + W1 quarters first) --------
            A = pool.tile([1, 1536], F32)          # [(c,k), r01*64]
            tv_sel = tv.rearrange("c (k s) w -> c k (s w)", s=16)[:, :, 7 * 64:9 * 64]
            nc.sync.dma_start(A[:].rearrange("p (c k x) -> p c k x", c=3, k=4),
                              tv_sel.unsqueeze(0))

            # W1 quarter q covers gates j in [16q, 16q+16) per partition
            W1sb = pool.tile([P, 64 * 48], F32)
            w1v = w1.rearrange("(p j) k -> p (j k)", p=P)
            for q in range(NQ):
                js = slice(q * JPQ * 48, (q + 1) * JPQ * 48)
                nc.sync.dma_start(W1sb[:, js], w1v[:, js])

            b1sb = pool.tile([P, 64], F32)
            nc.scalar.dma_start(b1sb[:], b1.rearrange("(p j) -> p j", p=P))
            b2sb = pool.tile([1, NCLS], F32)
            nc.scalar.dma_start(b2sb[:], b2.unsqueeze(0))
            W2sb = pool.tile([P, NCLS * 1536], F16)  # [p, c*1536+x] = W2p[c, p*1536+x]
            nc.scalar.dma_start(W2sb[:].rearrange("p (c x) -> p c x", c=NCLS),
                                w2.rearrange("c (p x) -> p c x", p=P))

            # ---- topview 64x64 -> 4x4 resize, flatten, scale ---------------
            V = pool.tile([1, 768], F32)
            A4 = A[:].rearrange("p (ck r w) -> p ck r w", ck=12, r=2)
            nc.vector.tensor_add(V[:].rearrange("p (ck w) -> p ck w", ck=12),
                                 A4[:, :, 0, :], A4[:, :, 1, :])
            F48 = pool.tile([1, 48], F32)
            V4 = V[:].rearrange("p (ck g s) -> p ck g s", ck=12, g=4)
            nc.vector.tensor_add(F48[:].rearrange("p (ck g) -> p ck g", ck=12),
                                 V4[:, :, :, 7], V4[:, :, :, 8])
            flatF = pool.tile([1, 48], F32)
            nc.vector.tensor_scalar_mul(flatF[:], F48[:], 0.25)

            # broadcast flat to all partitions via PE outer product with ones
            ppool = ctx.enter_context(tc.tile_pool(name="ps", bufs=1, space="PSUM"))
            ones1 = pool.tile([1, P], F32)
            nc.vector.memset(ones1[:], 1.0)
            flatb = ppool.tile([P, 48], F32)
            nc.tensor.matmul(out=flatb[:], lhsT=ones1[:], rhs=flatF[:],
                             start=True, stop=True)

            # ---- per-quarter: coords -> corners -> indices -> gathers ------
            mul1 = pool.tile([P, 64 * 48], F32)
            pre = pool.tile([P, 64], F32)
            sg = pool.tile([P, 64], F32)
            tl64 = pool.tile([P, 64], F32)
            rnd64 = pool.tile([P, 64], F32)
            gt64 = pool.tile([P, 64], F32)
            c064 = pool.tile([P, 64], F32)
            fr64 = pool.tile([P, 64], F32)
            fr16 = pool.tile([P, 64], F16)
            bq = pool.tile([P, TPP], F32)
            rndb = pool.tile([P, TPP], F32)
            gtb = pool.tile([P, TPP], F32)
            bf = pool.tile([P, TPP], F32)
            t2 = pool.tile([P, TPP], F32)
            t1 = pool.tile([P, TPP], F32)
            idxf = pool.tile([P, TPP], F32)
            idxi = pool.tile([P, TPP], I32)

            S = pool.tile([P, TPP * SEG], F16)
            D1 = pool.tile([P, TPP * 60], F16)
            M1 = pool.tile([P, TPP * 60], F16)
            T = pool.tile([P, TPP * 60], F16)
            D2 = pool.tile([P, TPP * 48], F16)
            M2 = pool.tile([P, TPP * 48], F16)
            U = pool.tile([P, TPP * 48], F16)
            prod = pool.tile([P, NCLS * 1536], F16)
            r2cls = pool.tile([P, NCLS * NQ], F32)

            Sv = S[:].rearrange("p (t x) -> p t x", t=TPP)
            W2v = W2sb[:].rearrange("p (c t x) -> p c t x", c=NCLS, t=TPP)
            prodv = prod[:].rearrange("p (c t x) -> p c t x", c=NCLS, t=TPP)
            fr2 = fr16[:].rearrange("p (t two) -> p t two", two=2)

            def emit_quarter(q):
                jsl = slice(q * JPQ, (q + 1) * JPQ)           # gate slots
                jse = slice(q * JPQ * 48, (q + 1) * JPQ * 48)
                tsl = slice(q * TPC, (q + 1) * TPC)           # patch slots
                nc.vector.tensor_mul(
                    mul1[:, jse].rearrange("p (j k) -> p j k", j=JPQ),
                    W1sb[:, jse].rearrange("p (j k) -> p j k", j=JPQ),
                    flatb[:].unsqueeze(1).to_broadcast((P, JPQ, 48)))
                nc.vector.reduce_sum(
                    pre[:, jsl].unsqueeze(2),
                    mul1[:, jse].rearrange("p (j k) -> p j k", j=JPQ),
                    axis=AX.X)
                nc.vector.tensor_add(pre[:, jsl], pre[:, jsl], b1sb[:, jsl])
                nc.scalar.activation(sg[:, jsl], pre[:, jsl], ACT.Sigmoid)

                # corners: tl = sg*pad; c0 = floor(tl) via round+correction
                nc.vector.tensor_scalar_mul(tl64[:, jsl], sg[:, jsl], pad)
                nc.vector.tensor_scalar(rnd64[:, jsl], tl64[:, jsl], MAGIC, MAGIC,
                                        op0=ALU.add, op1=ALU.subtract)
                nc.vector.tensor_tensor(gt64[:, jsl], rnd64[:, jsl], tl64[:, jsl],
                                        op=ALU.is_gt)
                nc.vector.tensor_sub(c064[:, jsl], rnd64[:, jsl], gt64[:, jsl])

                c2 = c064[:, jsl].rearrange("p (t two) -> p t two", two=2)
                r0f, c0f = c2[:, :, 0], c2[:, :, 1]
                # band = floor(r0/4); idx = band*16380 + r0 + 8*c0
                nc.vector.tensor_scalar_mul(bq[:, tsl], r0f, 0.25)
                nc.vector.tensor_scalar(rndb[:, tsl], bq[:, tsl], MAGIC, MAGIC,
                                        op0=ALU.add, op1=ALU.subtract)
                nc.vector.tensor_tensor(gtb[:, tsl], rndb[:, tsl], bq[:, tsl],
                                        op=ALU.is_gt)
                nc.vector.tensor_sub(bf[:, tsl], rndb[:, tsl], gtb[:, tsl])
                nc.vector.scalar_tensor_tensor(t2[:, tsl], c0f, 8.0, r0f,
                                               op0=ALU.mult, op1=ALU.add)
                nc.vector.tensor_scalar(t1[:, tsl], bf[:, tsl],
                                        float(BANDTRIP - 4), MAGIC,
                                        op0=ALU.mult, op1=ALU.add)
                nc.vector.tensor_add(idxf[:, tsl], t1[:, tsl], t2[:, tsl])
                nc.vector.tensor_single_scalar(idxi[:, tsl],
                                               idxf[:, tsl].bitcast(I32),
                                               0x007FFFFF, op=ALU.bitwise_and)
                # gather train for this quarter's 8 patch slots
                for t in range(q * TPC, (q + 1) * TPC):
                    nc.gpsimd.indirect_dma_start(
                        out=S[:, t * SEG:t * SEG + SEG],
                        out_offset=None,
                        in_=svb,
                        in_offset=IndirectOffsetOnAxis(ap=idxi[:, t:t + 1], axis=0),
                    )
                # fractions (consumed by this quarter's bilinear later)
                nc.vector.tensor_sub(fr64[:, jsl], tl64[:, jsl], c064[:, jsl])
                nc.vector.tensor_scalar_mul(fr16[:, jsl], fr64[:, jsl], 1.0)

            def emit_chunk(c):
                ts = slice(c * TPC, (c + 1) * TPC)

                def seg_view(off):
                    return Sv[:, ts, off:off + 120].rearrange(
                        "p t (d e) -> p t d e", d=5)[:, :, :, 0:12]

                frb = fr2[:, ts, 0].unsqueeze(2).unsqueeze(3).to_broadcast(
                    (P, TPC, 5, 12))
                fcb = fr2[:, ts, 1].unsqueeze(2).unsqueeze(3).to_broadcast(
                    (P, TPC, 4, 12))

                def c60(t_):
                    return t_[:, c * TPC * 60:(c + 1) * TPC * 60].rearrange(
                        "p (t d e) -> p t d e", t=TPC, d=5)

                def c48(t_):
                    return t_[:, c * TPC * 48:(c + 1) * TPC * 48].rearrange(
                        "p (t d e) -> p t d e", t=TPC, d=4)

                # row interp: T[d, i] = S[d, i] + fr*(S[d, i+1] - S[d, i])
                nc.vector.tensor_sub(c60(D1), seg_view(3), seg_view(0))
                nc.vector.tensor_mul(c60(M1), c60(D1), frb)
                nc.vector.tensor_add(c60(T), c60(M1), seg_view(0))
                # col interp
                Tc = T[:, c * TPC * 60:(c + 1) * TPC * 60].rearrange(
                    "p (t x) -> p t x", t=TPC)
                T0 = Tc[:, :, 0:48].rearrange("p t (d e) -> p t d e", d=4)
                T12 = Tc[:, :, 12:60].rearrange("p t (d e) -> p t d e", d=4)
                nc.vector.tensor_sub(c48(D2), T12, T0)
                nc.vector.tensor_mul(c48(M2), c48(D2), fcb)
                nc.vector.tensor_add(c48(U), c48(M2), T0)
                # classifier partial: fused mul + per-partition accumulate
                Uc = U[:, c * TPC * 48:(c + 1) * TPC * 48]
                for k in range(NCLS):
                    nc.vector.scalar_tensor_tensor(
                        prodv[:, k, ts, :].rearrange("p t x -> p (t x)"),
                        W2v[:, k, ts, :].rearrange("p t x -> p (t x)"),
                        1.0,
                        Uc,
                        op0=ALU.mult,
                        op1=ALU.mult,
                        accum_out=r2cls[:, k * NQ + c:k * NQ + c + 1],
                    )

            for q in range(NQ):
                emit_quarter(q)
            if debug:
                nc.sync.dma_start(dbg["idx"], idxi[:])
                nc.sync.dma_start(dbg["S"], S[:])
            for c in range(NQ):
                emit_chunk(c)

            # ---- finalize: sum chunks, reduce over partitions, + b2 --------
            r2 = pool.tile([P, NCLS], F32)
            nc.vector.reduce_sum(r2[:].unsqueeze(2),
                                 r2cls[:].rearrange("p (c n) -> p c n", c=NCLS),
                                 axis=AX.X)
            ones = pool.tile([P, 1], F32)
            nc.vector.memset(ones[:], 1.0)
            osum = ppool.tile([1, NCLS], F32)
            nc.tensor.matmul(out=osum[:], lhsT=ones[:], rhs=r2[:],
                             start=True, stop=True)
            ofin = pool.tile([1, NCLS], F32)
            nc.vector.tensor_add(ofin[:], osum[:], b2sb[:])
            nc.sync.dma_start(out, ofin[:])

    nc.compile()
    return nc


@functools.lru_cache(maxsize=2)
def _compiled(num_devices: int, svh: int, svw: int, debug: bool = False):
    return build_program(num_devices, svh, svw, debug)


def band_layout(img: np.ndarray) -> np.ndarray:
    """[2048, 2048, 3] f32 -> [511*16384 + pad, 3] fp16 bands."""
    sw = np.lib.stride_tricks.sliding_window_view(img, 8, axis=0)  # [2041, 2048, 3, 8]
    sb = sw[::4]                                                   # [511, 2048, 3, 8]
    flat = np.ascontiguousarray(
        sb.transpose(0, 1, 3, 2), dtype=np.float16).reshape(-1, 3)
    return np.concatenate([flat, np.zeros((PADTRIP, 3), np.float16)], axis=0)


def permute_w2(W2: np.ndarray) -> np.ndarray:
    """Reorder per-patch (i, j, c) -> (j, i, c) to match the kernel's U layout."""
    return np.ascontiguousarray(
        W2.reshape(NCLS, N, PS, PS, 3).transpose(0, 1, 3, 2, 4)).reshape(NCLS, -1)


def make_in_maps(topview, search_views, W1, b1, W2, b2):
    W1 = np.ascontiguousarray(W1, np.float32)
    b1 = np.ascontiguousarray(b1, np.float32)
    W2p = permute_w2(np.asarray(W2, np.float32)).astype(np.float16)
    b2 = np.ascontiguousarray(b2, np.float32)
    return [{
        "tv": np.ascontiguousarray(topview[i], np.float32),
        "svb": band_layout(np.asarray(search_views[i], np.float32)),
        "W1": W1, "b1": b1, "W2p": W2p, "b2": b2,
    } for i in range(topview.shape[0])]


def kernel(topview, search_views, W1, b1, W2, b2, svh, svw):
    svh, svw = int(svh), int(svw)
    nc = _compiled(B, svh, svw)
    in_maps = make_in_maps(topview, search_views, W1, b1, W2, b2)
    res = bass_utils.run_bass_kernel_spmd(nc, in_maps, core_ids=list(range(B)))
    return np.concatenate([res.results[i]["out"] for i in range(B)], axis=0)


# revision 20
# speedup vs baseline: 1.0501x; 1.0131x over previous
"""Trainium2 Bass kernel for nn_DZSpecimenClfToy.

Reference computation (per batch item b, B=8, one NeuronCore each):
  1. tv = bilinear_resize(topview[b], (3,64,64) -> (3,4,4))
  2. coords = sigmoid(tv.flat @ W1.T + b1).reshape(N,2)       # N=4096
  3. 4x4 patches bilinearly sampled at coords*2043 (5x5 px support)
  4. out[b] = patches.flat @ W2.T + b2                        # [2]

Sharding: data-parallel over batch across 8 cores; weights replicated.

Gather: host uploads the search view (fp16) in an overlapped-band layout
(511 bands of 8 rows at stride 4, each [col][row][ch]), so a patch's
5x5x3 support is one contiguous run at triple index b*16384 + c0*8 + s.
The HW indirect DMA consumes ONE offset per partition per instruction,
so the 4096 patches take 32 gathers of [128 x SEG]. The coords pipeline
is split in quarters so the gather train starts as early as possible,
and the bilinear + classifier (fp16, fused mul+accum) hide under it.
"""
import functools
from contextlib import ExitStack

import numpy as np

import concourse.bass as bass
import concourse.tile as tile
from concourse import bacc, mybir
import concourse.bass_utils as bass_utils
from concourse.bass import IndirectOffsetOnAxis

F32 = mybir.dt.float32
F16 = mybir.dt.float16
I32 = mybir.dt.int32
ALU = mybir.AluOpType
ACT = mybir.ActivationFunctionType
AX = mybir.AxisListType

B = 8          # batch == number of cores
H = W = 2048   # search view height/width
N = 4096       # patches per item
PS = 4         # patch size
NCLS = 2       # classes
P = 128        # partitions
TPP = N // P   # patches per partition = 32

NBAND = 511            # bands of 8 rows, stride 4: rows 4b..4b+7
BANDTRIP = W * 8       # pixel-triples per band = 16384
PADTRIP = 128          # svb tail pad: SEG overreads past the last band
SEG = 128              # gathered fp16 elements per patch (111 used)
MAGIC = 8388608.0      # 2**23

NQ = 4                 # coords pipeline quarters == gather/compute chunks
TPC = TPP // NQ        # patches per partition per chunk = 8
JPQ = 64 // NQ         # gates per partition per quarter = 16


def build_program(num_devices: int, svh: int, svw: int, debug: bool = False):
    pad = float(svh - 1 - PS)  # 2043
    assert svh == H and svw == W, (svh, svw)

    nc = bacc.Bacc("TRN2", target_bir_lowering=False, debug=False,
                   enable_asserts=False, num_devices=num_devices)

    tv = nc.dram_tensor("tv", [3, 64, 64], F32, kind="ExternalInput").ap()
    svb = nc.dram_tensor("svb", [NBAND * BANDTRIP + PADTRIP, 3], F16,
                         kind="ExternalInput").ap()
    w1 = nc.dram_tensor("W1", [2 * N, 48], F32, kind="ExternalInput").ap()
    b1 = nc.dram_tensor("b1", [2 * N], F32, kind="ExternalInput").ap()
    w2 = nc.dram_tensor("W2p", [NCLS, N * PS * PS * 3], F16, kind="ExternalInput").ap()
    b2 = nc.dram_tensor("b2", [NCLS], F32, kind="ExternalInput").ap()
    out = nc.dram_tensor("out", [1, NCLS], F32, kind="ExternalOutput").ap()

    dbg = {}
    if debug:
        dbg["idx"] = nc.dram_tensor("dbg_idx", [P, TPP], I32, kind="ExternalOutput").ap()
        dbg["S"] = nc.dram_tensor("dbg_S", [P, TPP * SEG], F16, kind="ExternalOutput").ap()

    with tile.TileContext(nc) as tc:
        with ExitStack() as ctx:
            pool = ctx.enter_context(tc.tile_pool(name="main", bufs=1))

            # ---- input DMAs (order matters: tv + W1 quarters first) --------
            A = pool.tile([1, 1536], F32)          # [(c,k), r01*64]
            tv_sel = tv.rearrange("c (k s) w -> c k (s w)", s=16)[:, :, 7 * 64:9 * 64]
            nc.sync.dma_start(A[:].rearrange("p (c k x) -> p c k x", c=3, k=4),
                              tv_sel.unsqueeze(0))

            # W1 quarter q covers gates j in [16q, 16q+16) per partition
            W1sb = pool.tile([P, 64 * 48], F32)
            w1v = w1.rearrange("(p j) k -> p (j k)", p=P)
            for q in range(NQ):
                js = slice(q * JPQ * 48, (q + 1) * JPQ * 48)
                nc.sync.dma_start(W1sb[:, js], w1v[:, js])

            b1sb = pool.tile([P, 64], F32)
            nc.scalar.dma_start(b1sb[:], b1.rearrange("(p j) -> p j", p=P))
            b2sb = pool.tile([1, NCLS], F32)
            W2sb = pool.tile([P, NCLS * 1536], F16)  # [p, c*1536+x] = W2p[c, p*1536+x]

            # ---- topview 64x64 -> 4x4 resize, flatten, scale ---------------
            V = pool.tile([1, 768], F32)
            A4 = A[:].rearrange("p (ck r w) -> p ck r w", ck=12, r=2)
            nc.vector.tensor_add(V[:].rearrange("p (ck w) -> p ck w", ck=12),
                                 A4[:, :, 0, :], A4[:, :, 1, :])
            F48 = pool.tile([1, 48], F32)
            V4 = V[:].rearrange("p (ck g s) -> p ck g s", ck=12, g=4)
            nc.vector.tensor_add(F48[:].rearrange("p (ck g) -> p ck g", ck=12),
                                 V4[:, :, :, 7], V4[:, :, :, 8])
            flatF = pool.tile([1, 48], F32)
            nc.vector.tensor_scalar_mul(flatF[:], F48[:], 0.25)

            # broadcast flat to all partitions via PE outer product with ones
            ppool = ctx.enter_context(tc.tile_pool(name="ps", bufs=1, space="PSUM"))
            ones1 = pool.tile([1, P], F32)
            nc.vector.memset(ones1[:], 1.0)
            flatb = ppool.tile([P, 48], F32)
            nc.tensor.matmul(out=flatb[:], lhsT=ones1[:], rhs=flatF[:],
                             start=True, stop=True)

            # ---- per-quarter: coords -> corners -> indices -> gathers ------
            mul1 = pool.tile([P, 64 * 48], F32)
            pre = pool.tile([P, 64], F32)
            sg = pool.tile([P, 64], F32)
            tl64 = pool.tile([P, 64], F32)
            rnd64 = pool.tile([P, 64], F32)
            gt64 = pool.tile([P, 64], F32)
            c064 = pool.tile([P, 64], F32)
            fr64 = pool.tile([P, 64], F32)
            fr16 = pool.tile([P, 64], F16)
            bq = pool.tile([P, TPP], F32)
            rndb = pool.tile([P, TPP], F32)
            gtb = pool.tile([P, TPP], F32)
            bf = pool.tile([P, TPP], F32)
            t2 = pool.tile([P, TPP], F32)
            t1 = pool.tile([P, TPP], F32)
            idxf = pool.tile([P, TPP], F32)
            idxi = pool.tile([P, TPP], I32)

            S = pool.tile([P, TPP * SEG], F16)
            D1 = pool.tile([P, TPP * 60], F16)
            M1 = pool.tile([P, TPP * 60], F16)
            T = pool.tile([P, TPP * 60], F16)
            D2 = pool.tile([P, TPP * 48], F16)
            M2 = pool.tile([P, TPP * 48], F16)
            U = pool.tile([P, TPP * 48], F16)
            prod = pool.tile([P, NCLS * 1536], F16)
            r2cls = pool.tile([P, NCLS * NQ], F32)

            Sv = S[:].rearrange("p (t x) -> p t x", t=TPP)
            W2v = W2sb[:].rearrange("p (c t x) -> p c t x", c=NCLS, t=TPP)
            prodv = prod[:].rearrange("p (c t x) -> p c t x", c=NCLS, t=TPP)
            fr2 = fr16[:].rearrange("p (t two) -> p t two", two=2)

            def emit_quarter(q):
                jsl = slice(q * JPQ, (q + 1) * JPQ)           # gate slots
                jse = slice(q * JPQ * 48, (q + 1) * JPQ * 48)
                tsl = slice(q * TPC, (q + 1) * TPC)           # patch slots
                # corner/idx chain for q=0 runs on the (still idle) Pool
                # engine so the gather train starts before the scheduler
                # works through the other quarters' DVE mul/reduce ops.
                eng = nc.vector
                nc.vector.tensor_mul(
                    mul1[:, jse].rearrange("p (j k) -> p j k", j=JPQ),
                    W1sb[:, jse].rearrange("p (j k) -> p j k", j=JPQ),
                    flatb[:].unsqueeze(1).to_broadcast((P, JPQ, 48)))
                nc.vector.reduce_sum(
                    pre[:, jsl].unsqueeze(2),
                    mul1[:, jse].rearrange("p (j k) -> p j k", j=JPQ),
                    axis=AX.X)
                nc.vector.tensor_add(pre[:, jsl], pre[:, jsl], b1sb[:, jsl])
                nc.scalar.activation(sg[:, jsl], pre[:, jsl], ACT.Sigmoid)

                # corners: tl = sg*pad; c0 = floor(tl) via round+correction
                eng.tensor_scalar_mul(tl64[:, jsl], sg[:, jsl], pad)
                eng.tensor_scalar(rnd64[:, jsl], tl64[:, jsl], MAGIC, MAGIC,
                                  op0=ALU.add, op1=ALU.subtract)
                eng.tensor_tensor(gt64[:, jsl], rnd64[:, jsl], tl64[:, jsl],
                                  op=ALU.is_gt)
                eng.tensor_sub(c064[:, jsl], rnd64[:, jsl], gt64[:, jsl])

                c2 = c064[:, jsl].rearrange("p (t two) -> p t two", two=2)
                r0f, c0f = c2[:, :, 0], c2[:, :, 1]
                # band = floor(r0/4); idx = band*16380 + r0 + 8*c0
                eng.tensor_scalar_mul(bq[:, tsl], r0f, 0.25)
                eng.tensor_scalar(rndb[:, tsl], bq[:, tsl], MAGIC, MAGIC,
                                  op0=ALU.add, op1=ALU.subtract)
                eng.tensor_tensor(gtb[:, tsl], rndb[:, tsl], bq[:, tsl],
                                  op=ALU.is_gt)
                eng.tensor_sub(bf[:, tsl], rndb[:, tsl], gtb[:, tsl])
                eng.scalar_tensor_tensor(t2[:, tsl], c0f, 8.0, r0f,
                                         op0=ALU.mult, op1=ALU.add)
                eng.tensor_scalar(t1[:, tsl], bf[:, tsl],
                                  float(BANDTRIP - 4), MAGIC,
                                  op0=ALU.mult, op1=ALU.add)
                eng.tensor_add(idxf[:, tsl], t1[:, tsl], t2[:, tsl])
                eng.tensor_single_scalar(idxi[:, tsl],
                                         idxf[:, tsl].bitcast(I32),
                                         0x007FFFFF, op=ALU.bitwise_and)
                # gather train for this quarter's 8 patch slots
                for t in range(q * TPC, (q + 1) * TPC):
                    nc.gpsimd.indirect_dma_start(
                        out=S[:, t * SEG:t * SEG + SEG],
                        out_offset=None,
                        in_=svb,
                        in_offset=IndirectOffsetOnAxis(ap=idxi[:, t:t + 1], axis=0),
                    )
                # fractions (consumed by this quarter's bilinear later)
                nc.vector.tensor_sub(fr64[:, jsl], tl64[:, jsl], c064[:, jsl])
                nc.vector.tensor_scalar_mul(fr16[:, jsl], fr64[:, jsl], 1.0)
                if q == 0:
                    # W2/b2 loads issued only now: keeps early HBM bandwidth
                    # for the W1 quarters feeding the critical path.
                    nc.scalar.dma_start(b2sb[:], b2.unsqueeze(0))
                    nc.scalar.dma_start(
                        W2sb[:].rearrange("p (c x) -> p c x", c=NCLS),
                        w2.rearrange("c (p x) -> p c x", p=P))

            def emit_chunk(c):
                ts = slice(c * TPC, (c + 1) * TPC)

                def seg_view(off):
                    return Sv[:, ts, off:off + 120].rearrange(
                        "p t (d e) -> p t d e", d=5)[:, :, :, 0:12]

                frb = fr2[:, ts, 0].unsqueeze(2).unsqueeze(3).to_broadcast(
                    (P, TPC, 5, 12))
                fcb = fr2[:, ts, 1].unsqueeze(2).unsqueeze(3).to_broadcast(
                    (P, TPC, 4, 12))

                def c60(t_):
                    return t_[:, c * TPC * 60:(c + 1) * TPC * 60].rearrange(
                        "p (t d e) -> p t d e", t=TPC, d=5)

                def c48(t_):
                    return t_[:, c * TPC * 48:(c + 1) * TPC * 48].rearrange(
                        "p (t d e) -> p t d e", t=TPC, d=4)

                # row interp: T[d, i] = S[d, i] + fr*(S[d, i+1] - S[d, i])
                nc.vector.tensor_sub(c60(D1), seg_view(3), seg_view(0))
                nc.vector.tensor_mul(c60(M1), c60(D1), frb)
                nc.vector.tensor_add(c60(T), c60(M1), seg_view(0))
                # col interp
                Tc = T[:, c * TPC * 60:(c + 1) * TPC * 60].rearrange(
                    "p (t x) -> p t x", t=TPC)
                T0 = Tc[:, :, 0:48].rearrange("p t (d e) -> p t d e", d=4)
                T12 = Tc[:, :, 12:60].rearrange("p t (d e) -> p t d e", d=4)
                nc.vector.tensor_sub(c48(D2), T12, T0)
                nc.vector.tensor_mul(c48(M2), c48(D2), fcb)
                nc.vector.tensor_add(c48(U), c48(M2), T0)
                # classifier partial: fused mul + per-partition accumulate
                Uc = U[:, c * TPC * 48:(c + 1) * TPC * 48]
                for k in range(NCLS):
                    nc.vector.scalar_tensor_tensor(
                        prodv[:, k, ts, :].rearrange("p t x -> p (t x)"),
                        W2v[:, k, ts, :].rearrange("p t x -> p (t x)"),
                        1.0,
                        Uc,
                        op0=ALU.mult,
                        op1=ALU.mult,
                        accum_out=r2cls[:, k * NQ + c:k * NQ + c + 1],
                    )

            for q in range(NQ):
                emit_quarter(q)
            if debug:
                nc.sync.dma_start(dbg["idx"], idxi[:])
                nc.sync.dma_start(dbg["S"], S[:])
            for c in range(NQ):
                emit_chunk(c)

            # ---- finalize: sum chunks, reduce over partitions, + b2 --------
            r2 = pool.tile([P, NCLS], F32)
            nc.vector.reduce_sum(r2[:].unsqueeze(2),
                                 r2cls[:].rearrange("p (c n) -> p c n", c=NCLS),
                                 axis=AX.X)
            ones = pool.tile([P, 1], F32)
            nc.vector.memset(ones[:], 1.0)
            osum = ppool.tile([1, NCLS], F32)
            nc.tensor.matmul(out=osum[:], lhsT=ones[:], rhs=r2[:],
                             start=True, stop=True)
            ofin = pool.tile([1, NCLS], F32)
            nc.vector.tensor_add(ofin[:], osum[:], b2sb[:])
            nc.sync.dma_start(out, ofin[:])

    nc.compile()
    return nc


@functools.lru_cache(maxsize=2)
def _compiled(num_devices: int, svh: int, svw: int, debug: bool = False):
    return build_program(num_devices, svh, svw, debug)


def band_layout(img: np.ndarray) -> np.ndarray:
    """[2048, 2048, 3] f32 -> [511*16384 + pad, 3] fp16 bands."""
    sw = np.lib.stride_tricks.sliding_window_view(img, 8, axis=0)  # [2041, 2048, 3, 8]
    sb = sw[::4]                                                   # [511, 2048, 3, 8]
    flat = np.ascontiguousarray(
        sb.transpose(0, 1, 3, 2), dtype=np.float16).reshape(-1, 3)
    return np.concatenate([flat, np.zeros((PADTRIP, 3), np.float16)], axis=0)


def permute_w2(W2: np.ndarray) -> np.ndarray:
    """Reorder per-patch (i, j, c) -> (j, i, c) to match the kernel's U layout."""
    return np.ascontiguousarray(
        W2.reshape(NCLS, N, PS, PS, 3).transpose(0, 1, 3, 2, 4)).reshape(NCLS, -1)


def make_in_maps(topview, search_views, W1, b1, W2, b2):
    W1 = np.ascontiguousarray(W1, np.float32)
    b1 = np.ascontiguousarray(b1, np.float32)
    W2p = permute_w2(np.asarray(W2, np.float32)).astype(np.float16)
    b2 = np.ascontiguousarray(b2, np.float32)
    return [{
        "tv": np.ascontiguousarray(topview[i], np.float32),
        "svb": band_layout(np.asarray(search_views[i], np.float32)),
        "W1": W1, "b1": b1, "W2p": W2p, "b2": b2,
    } for i in range(topview.shape[0])]


def kernel(topview, search_views, W1, b1, W2, b2, svh, svw):
    svh, svw = int(svh), int(svw)
    nc = _compiled(B, svh, svw)
    in_maps = make_in_maps(topview, search_views, W1, b1, W2, b2)
    res = bass_utils.run_bass_kernel_spmd(nc, in_maps, core_ids=list(range(B)))
    return np.concatenate([res.results[i]["out"] for i in range(B)], axis=0)


# revision 21
# speedup vs baseline: 1.0644x; 1.0136x over previous
"""Trainium2 Bass kernel for nn_DZSpecimenClfToy.

Reference computation (per batch item b, B=8, one NeuronCore each):
  1. tv = bilinear_resize(topview[b], (3,64,64) -> (3,4,4))
  2. coords = sigmoid(tv.flat @ W1.T + b1).reshape(N,2)       # N=4096
  3. 4x4 patches bilinearly sampled at coords*2043 (5x5 px support)
  4. out[b] = patches.flat @ W2.T + b2                        # [2]

Sharding: data-parallel over batch across 8 cores; weights replicated.

Gather: host uploads the search view (fp16) in an overlapped-band layout
(511 bands of 8 rows at stride 4, each [col][row][ch]), so a patch's
5x5x3 support is one contiguous run at triple index b*16384 + c0*8 + s.
The HW indirect DMA consumes ONE offset per partition per instruction,
so the 4096 patches take 32 gathers of [128 x SEG]. The coords pipeline
is split in quarters so the gather train starts as early as possible,
and the bilinear + classifier (fp16, fused mul+accum) hide under it.
"""
import functools
from contextlib import ExitStack

import numpy as np

import concourse.bass as bass
import concourse.tile as tile
from concourse import bacc, mybir
import concourse.bass_utils as bass_utils
from concourse.bass import IndirectOffsetOnAxis

F32 = mybir.dt.float32
F16 = mybir.dt.float16
I32 = mybir.dt.int32
ALU = mybir.AluOpType
ACT = mybir.ActivationFunctionType
AX = mybir.AxisListType

B = 8          # batch == number of cores
H = W = 2048   # search view height/width
N = 4096       # patches per item
PS = 4         # patch size
NCLS = 2       # classes
P = 128        # partitions
TPP = N // P   # patches per partition = 32

NBAND = 511            # bands of 8 rows, stride 4: rows 4b..4b+7
BANDTRIP = W * 8       # pixel-triples per band = 16384
PADTRIP = 128          # svb tail pad: SEG overreads past the last band
SEG = 128              # gathered fp16 elements per patch (111 used)
MAGIC = 8388608.0      # 2**23

NQ = 8                 # coords pipeline stages == gather/compute chunks
TPC = TPP // NQ        # patches per partition per chunk = 4
JPQ = 64 // NQ         # gates per partition per stage = 8


def build_program(num_devices: int, svh: int, svw: int, debug: bool = False):
    pad = float(svh - 1 - PS)  # 2043
    assert svh == H and svw == W, (svh, svw)

    nc = bacc.Bacc("TRN2", target_bir_lowering=False, debug=False,
                   enable_asserts=False, num_devices=num_devices)

    tv = nc.dram_tensor("tv", [3, 64, 64], F32, kind="ExternalInput").ap()
    svb = nc.dram_tensor("svb", [NBAND * BANDTRIP + PADTRIP, 3], F16,
                         kind="ExternalInput").ap()
    w1 = nc.dram_tensor("W1", [2 * N, 48], F32, kind="ExternalInput").ap()
    b1 = nc.dram_tensor("b1", [2 * N], F32, kind="ExternalInput").ap()
    w2 = nc.dram_tensor("W2p", [NCLS, N * PS * PS * 3], F16, kind="ExternalInput").ap()
    b2 = nc.dram_tensor("b2", [NCLS], F32, kind="ExternalInput").ap()
    out = nc.dram_tensor("out", [1, NCLS], F32, kind="ExternalOutput").ap()

    dbg = {}
    if debug:
        dbg["idx"] = nc.dram_tensor("dbg_idx", [P, TPP], I32, kind="ExternalOutput").ap()
        dbg["S"] = nc.dram_tensor("dbg_S", [P, TPP * SEG], F16, kind="ExternalOutput").ap()

    with tile.TileContext(nc) as tc:
        with ExitStack() as ctx:
            pool = ctx.enter_context(tc.tile_pool(name="main", bufs=1))

            # ---- input DMAs (order matters: W1 stage 0 + tv first) ---------
            A = pool.tile([1, 1536], F32)          # [(c,k), r01*64]
            W1sb = pool.tile([P, 64 * 48], F32)
            w1v = w1.rearrange("(p j) k -> p (j k)", p=P)
            nc.sync.dma_start(W1sb[:, :JPQ * 48], w1v[:, :JPQ * 48])
            tv_sel = tv.rearrange("c (k s) w -> c k (s w)", s=16)[:, :, 7 * 64:9 * 64]
            nc.sync.dma_start(A[:].rearrange("p (c k x) -> p c k x", c=3, k=4),
                              tv_sel.unsqueeze(0))
            for q in range(1, NQ):
                js = slice(q * JPQ * 48, (q + 1) * JPQ * 48)
                nc.sync.dma_start(W1sb[:, js], w1v[:, js])

            b1sb = pool.tile([P, 64], F32)
            nc.scalar.dma_start(b1sb[:], b1.rearrange("(p j) -> p j", p=P))
            b2sb = pool.tile([1, NCLS], F32)
            W2sb = pool.tile([P, NCLS * 1536], F16)  # [p, c*1536+x] = W2p[c, p*1536+x]

            # ---- topview 64x64 -> 4x4 resize, flatten, scale ---------------
            V = pool.tile([1, 768], F32)
            A4 = A[:].rearrange("p (ck r w) -> p ck r w", ck=12, r=2)
            nc.vector.tensor_add(V[:].rearrange("p (ck w) -> p ck w", ck=12),
                                 A4[:, :, 0, :], A4[:, :, 1, :])
            flatF = pool.tile([1, 48], F32)
            V4 = V[:].rearrange("p (ck g s) -> p ck g s", ck=12, g=4)
            nc.vector.tensor_add(flatF[:].rearrange("p (ck g) -> p ck g", ck=12),
                                 V4[:, :, :, 7], V4[:, :, :, 8])

            # broadcast flat to all partitions via PE outer product; the
            # resize's 0.25 scale is folded into the broadcast vector.
            ppool = ctx.enter_context(tc.tile_pool(name="ps", bufs=1, space="PSUM"))
            ones1 = pool.tile([1, P], F32)
            nc.vector.memset(ones1[:], 0.25)
            flatb = ppool.tile([P, 48], F32)
            nc.tensor.matmul(out=flatb[:], lhsT=ones1[:], rhs=flatF[:],
                             start=True, stop=True)

            # ---- per-quarter: coords -> corners -> indices -> gathers ------
            mul1 = pool.tile([P, 64 * 48], F32)
            pre = pool.tile([P, 64], F32)
            sg = pool.tile([P, 64], F32)
            tl64 = pool.tile([P, 64], F32)
            rnd64 = pool.tile([P, 64], F32)
            gt64 = pool.tile([P, 64], F32)
            c064 = pool.tile([P, 64], F32)
            fr64 = pool.tile([P, 64], F32)
            fr16 = pool.tile([P, 64], F16)
            bq = pool.tile([P, TPP], F32)
            rndb = pool.tile([P, TPP], F32)
            gtb = pool.tile([P, TPP], F32)
            bf = pool.tile([P, TPP], F32)
            t2 = pool.tile([P, TPP], F32)
            t1 = pool.tile([P, TPP], F32)
            idxf = pool.tile([P, TPP], F32)
            idxi = pool.tile([P, TPP], I32)

            S = pool.tile([P, TPP * SEG], F16)
            D1 = pool.tile([P, TPP * 60], F16)
            M1 = pool.tile([P, TPP * 60], F16)
            T = pool.tile([P, TPP * 60], F16)
            D2 = pool.tile([P, TPP * 48], F16)
            M2 = pool.tile([P, TPP * 48], F16)
            U = pool.tile([P, TPP * 48], F16)
            prod = pool.tile([P, NCLS * 1536], F16)
            r2cls = pool.tile([P, NCLS * NQ], F32)

            Sv = S[:].rearrange("p (t x) -> p t x", t=TPP)
            W2v = W2sb[:].rearrange("p (c t x) -> p c t x", c=NCLS, t=TPP)
            prodv = prod[:].rearrange("p (c t x) -> p c t x", c=NCLS, t=TPP)
            fr2 = fr16[:].rearrange("p (t two) -> p t two", two=2)

            def emit_quarter(q):
                jsl = slice(q * JPQ, (q + 1) * JPQ)           # gate slots
                jse = slice(q * JPQ * 48, (q + 1) * JPQ * 48)
                tsl = slice(q * TPC, (q + 1) * TPC)           # patch slots
                # corner/idx chain for q=0 runs on the (still idle) Pool
                # engine so the gather train starts before the scheduler
                # works through the other quarters' DVE mul/reduce ops.
                eng = nc.vector
                nc.vector.tensor_mul(
                    mul1[:, jse].rearrange("p (j k) -> p j k", j=JPQ),
                    W1sb[:, jse].rearrange("p (j k) -> p j k", j=JPQ),
                    flatb[:].unsqueeze(1).to_broadcast((P, JPQ, 48)))
                nc.vector.reduce_sum(
                    pre[:, jsl].unsqueeze(2),
                    mul1[:, jse].rearrange("p (j k) -> p j k", j=JPQ),
                    axis=AX.X)
                nc.vector.tensor_add(pre[:, jsl], pre[:, jsl], b1sb[:, jsl])
                nc.scalar.activation(sg[:, jsl], pre[:, jsl], ACT.Sigmoid)

                # corners: tl = sg*pad; c0 = floor(tl) via round+correction
                eng.tensor_scalar_mul(tl64[:, jsl], sg[:, jsl], pad)
                eng.tensor_scalar(rnd64[:, jsl], tl64[:, jsl], MAGIC, MAGIC,
                                  op0=ALU.add, op1=ALU.subtract)
                eng.tensor_tensor(gt64[:, jsl], rnd64[:, jsl], tl64[:, jsl],
                                  op=ALU.is_gt)
                eng.tensor_sub(c064[:, jsl], rnd64[:, jsl], gt64[:, jsl])

                c2 = c064[:, jsl].rearrange("p (t two) -> p t two", two=2)
                r0f, c0f = c2[:, :, 0], c2[:, :, 1]
                # band = floor(r0/4); idx = band*16380 + r0 + 8*c0
                eng.tensor_scalar_mul(bq[:, tsl], r0f, 0.25)
                eng.tensor_scalar(rndb[:, tsl], bq[:, tsl], MAGIC, MAGIC,
                                  op0=ALU.add, op1=ALU.subtract)
                eng.tensor_tensor(gtb[:, tsl], rndb[:, tsl], bq[:, tsl],
                                  op=ALU.is_gt)
                eng.tensor_sub(bf[:, tsl], rndb[:, tsl], gtb[:, tsl])
                eng.scalar_tensor_tensor(t2[:, tsl], c0f, 8.0, r0f,
                                         op0=ALU.mult, op1=ALU.add)
                eng.tensor_scalar(t1[:, tsl], bf[:, tsl],
                                  float(BANDTRIP - 4), MAGIC,
                                  op0=ALU.mult, op1=ALU.add)
                eng.tensor_add(idxf[:, tsl], t1[:, tsl], t2[:, tsl])
                eng.tensor_single_scalar(idxi[:, tsl],
                                         idxf[:, tsl].bitcast(I32),
                                         0x007FFFFF, op=ALU.bitwise_and)
                # gather train for this quarter's 8 patch slots
                for t in range(q * TPC, (q + 1) * TPC):
                    nc.gpsimd.indirect_dma_start(
                        out=S[:, t * SEG:t * SEG + SEG],
                        out_offset=None,
                        in_=svb,
                        in_offset=IndirectOffsetOnAxis(ap=idxi[:, t:t + 1], axis=0),
                    )
                # fractions (consumed by this quarter's bilinear later)
                nc.vector.tensor_sub(fr64[:, jsl], tl64[:, jsl], c064[:, jsl])
                nc.vector.tensor_scalar_mul(fr16[:, jsl], fr64[:, jsl], 1.0)
                if q == 0:
                    # W2/b2 loads issued only now: keeps early HBM bandwidth
                    # for the W1 quarters feeding the critical path.
                    nc.scalar.dma_start(b2sb[:], b2.unsqueeze(0))
                    nc.scalar.dma_start(
                        W2sb[:].rearrange("p (c x) -> p c x", c=NCLS),
                        w2.rearrange("c (p x) -> p c x", p=P))

            def emit_chunk(c):
                ts = slice(c * TPC, (c + 1) * TPC)

                def seg_view(off):
                    return Sv[:, ts, off:off + 120].rearrange(
                        "p t (d e) -> p t d e", d=5)[:, :, :, 0:12]

                frb = fr2[:, ts, 0].unsqueeze(2).unsqueeze(3).to_broadcast(
                    (P, TPC, 5, 12))
                fcb = fr2[:, ts, 1].unsqueeze(2).unsqueeze(3).to_broadcast(
                    (P, TPC, 4, 12))

                def c60(t_):
                    return t_[:, c * TPC * 60:(c + 1) * TPC * 60].rearrange(
                        "p (t d e) -> p t d e", t=TPC, d=5)

                def c48(t_):
                    return t_[:, c * TPC * 48:(c + 1) * TPC * 48].rearrange(
                        "p (t d e) -> p t d e", t=TPC, d=4)

                # row interp: T[d, i] = S[d, i] + fr*(S[d, i+1] - S[d, i])
                nc.vector.tensor_sub(c60(D1), seg_view(3), seg_view(0))
                nc.vector.tensor_mul(c60(M1), c60(D1), frb)
                nc.vector.tensor_add(c60(T), c60(M1), seg_view(0))
                # col interp
                Tc = T[:, c * TPC * 60:(c + 1) * TPC * 60].rearrange(
                    "p (t x) -> p t x", t=TPC)
                T0 = Tc[:, :, 0:48].rearrange("p t (d e) -> p t d e", d=4)
                T12 = Tc[:, :, 12:60].rearrange("p t (d e) -> p t d e", d=4)
                nc.vector.tensor_sub(c48(D2), T12, T0)
                nc.vector.tensor_mul(c48(M2), c48(D2), fcb)
                nc.vector.tensor_add(c48(U), c48(M2), T0)
                # classifier partial: fused mul + per-partition accumulate
                Uc = U[:, c * TPC * 48:(c + 1) * TPC * 48]
                for k in range(NCLS):
                    nc.vector.scalar_tensor_tensor(
                        prodv[:, k, ts, :].rearrange("p t x -> p (t x)"),
                        W2v[:, k, ts, :].rearrange("p t x -> p (t x)"),
                        1.0,
                        Uc,
                        op0=ALU.mult,
                        op1=ALU.mult,
                        accum_out=r2cls[:, k * NQ + c:k * NQ + c + 1],
                    )

            for q in range(NQ):
                emit_quarter(q)
            if debug:
                nc.sync.dma_start(dbg["idx"], idxi[:])
                nc.sync.dma_start(dbg["S"], S[:])
            for c in range(NQ):
                emit_chunk(c)

            # ---- finalize: sum chunks, reduce over partitions, + b2 --------
            r2 = pool.tile([P, NCLS], F32)
            nc.vector.reduce_sum(r2[:].unsqueeze(2),
                                 r2cls[:].rearrange("p (c n) -> p c n", c=NCLS),
                                 axis=AX.X)
            ones = pool.tile([P, 1], F32)
            nc.vector.memset(ones[:], 1.0)
            osum = ppool.tile([1, NCLS], F32)
            nc.tensor.matmul(out=osum[:], lhsT=ones[:], rhs=r2[:],
                             start=True, stop=True)
            ofin = pool.tile([1, NCLS], F32)
            nc.vector.tensor_add(ofin[:], osum[:], b2sb[:])
            nc.sync.dma_start(out, ofin[:])

    nc.compile()
    return nc


@functools.lru_cache(maxsize=2)
def _compiled(num_devices: int, svh: int, svw: int, debug: bool = False):
    return build_program(num_devices, svh, svw, debug)


def band_layout(img: np.ndarray) -> np.ndarray:
    """[2048, 2048, 3] f32 -> [511*16384 + pad, 3] fp16 bands."""
    sw = np.lib.stride_tricks.sliding_window_view(img, 8, axis=0)  # [2041, 2048, 3, 8]
    sb = sw[::4]                                                   # [511, 2048, 3, 8]
    flat = np.ascontiguousarray(
        sb.transpose(0, 1, 3, 2), dtype=np.float16).reshape(-1, 3)
    return np.concatenate([flat, np.zeros((PADTRIP, 3), np.float16)], axis=0)


def permute_w2(W2: np.ndarray) -> np.ndarray:
    """Reorder per-patch (i, j, c) -> (j, i, c) to match the kernel's U layout."""
    return np.ascontiguousarray(
        W2.reshape(NCLS, N, PS, PS, 3).transpose(0, 1, 3, 2, 4)).reshape(NCLS, -1)


def make_in_maps(topview, search_views, W1, b1, W2, b2):
    W1 = np.ascontiguousarray(W1, np.float32)
    b1 = np.ascontiguousarray(b1, np.float32)
    W2p = permute_w2(np.asarray(W2, np.float32)).astype(np.float16)
    b2 = np.ascontiguousarray(b2, np.float32)
    return [{
        "tv": np.ascontiguousarray(topview[i], np.float32),
        "svb": band_layout(np.asarray(search_views[i], np.float32)),
        "W1": W1, "b1": b1, "W2p": W2p, "b2": b2,
    } for i in range(topview.shape[0])]


def kernel(topview, search_views, W1, b1, W2, b2, svh, svw):
    svh, svw = int(svh), int(svw)
    nc = _compiled(B, svh, svw)
    in_maps = make_in_maps(topview, search_views, W1, b1, W2, b2)
    res = bass_utils.run_bass_kernel_spmd(nc, in_maps, core_ids=list(range(B)))
    return np.concatenate([res.results[i]["out"] for i in range(B)], axis=0)
